# revision 38
# baseline (speedup 1.0000x reference)
"""Trainium2 Bass kernel for nn_EnhancedQuanvolution (v3).

Computes, for x [B,1,28,28] f32, W [10,784], b [10]:
    per 2x2 patch p of the 28x28 image, ez[:, p, j] = cumprod_j cos(patch vals)
    logits = ez.reshape(B,784) @ W.T + b ;  out = log_softmax(logits)

v2 core design (see git-less history): host ships a = wrap(x + pi/2) in
[-pi, pi] as bf16, permuted into parity-plane order [pl(4), r(14), c(14)];
cos x = sin(a) -> one contiguous in-place Sin per macro-tile on ACT.
Cumprod muls column-split DVE/Pool at the balance point; PSUM->SBUF copies
of PE-transposed features ride DVE 2x_1P; per-PSUM-bank log-softmax tails;
partition-major sample order for contiguous output DMA runs.

v3 deltas (all verified legal on the walrus/device path):
- no DMA on the ACT HWDGE ring before the first Sin: kills a spurious
  exp_and_others table load (fill -1.28us); fill DMAs ride Pool SWDGE.
- output DMAs moved to the SP ring (idle at drain).
- last pair-copy on ACT (idle after the final Sin), no tail taper,
  logits banks (51,13) (51*10*4B = 2040B fills a PSUM bank; small last
  bank shortens the final softmax tail), dve_mul_cols 88.
- bias folded into the matmul accumulation (1^T @ b opens each group's
  chain): no DVE bias-add pass; Exp and the final subtract read logits
  straight from PSUM.  Per-bank softmax chains (bank_chain) emitted
  exp->reduce->ln->sub->dma per bank so bank0's ln never queues behind
  bank1's dep-blocked exp in the ACT FIFO.
Dead ends kept as documented options, OFF: stt_pool (TensorScalarPtr is
rejected on Pool by walrus codegen: "Instruction engine check failed"),
mm_lag (deferred-matmul emission produces nondeterministically wrong
results in multi-core runs), pool_reduce (Pool reduce is partition-axis
only), dma_copy_every (DMA cannot read PSUM: bass asserts).
Engine budget (TimelineSim): window = ACT Sin stream 46.6us (41.8 pure
Sin floor + per-instr SBUF-access busy), Pool ~48 (muls at 0.42 gpsimd
efficiency + Q7 launches), DVE ~46 (copies 34 + mul share + softmax),
DMA 36.9, PE 26.5.  TimelineSim 63735 ns -> calibrated HW 69064 ns.
rel err vs reference: 0.0023559 (< 2e-2 gate), deterministic across runs.
"""
import sys

sys.path.insert(0, "/opt/trn_rl_repo")

import numpy as np
import ml_dtypes
from contextlib import ExitStack

import concourse.bass as bass
import concourse.tile as tile
from concourse import bacc, mybir
from concourse.bass_utils import run_bass_kernel_spmd
import concourse.hw_specs as hw_specs

# Make the act-table chooser resolve Exp and Ln to the one set that holds
# both (natural_log_exp_and_others): 2 table loads total instead of 3, and a
# dummy Exp after the last Sin prefetches the tail's set off the critical
# path.  Only the chooser is filtered — the runtime tables are unchanged.
_orig_get_tables = hw_specs.get_activation_tables
_EXP = mybir.ActivationFunctionType.Exp
_LN = mybir.ActivationFunctionType.Ln


def _filtered_tables(arch):
    tabs = dict(_orig_get_tables(arch))
    for name, fns in list(tabs.items()):
        if name != "natural_log_exp_and_others" and (_EXP in fns or _LN in fns):
            tabs[name] = fns - {_EXP, _LN}
    return tabs


for _mod in (hw_specs, bacc):
    if getattr(_mod, "get_activation_tables", None) is _orig_get_tables:
        _mod.get_activation_tables = _filtered_tables

F32 = mybir.dt.float32
BF16 = mybir.dt.bfloat16
AF = mybir.ActivationFunctionType
PI = float(np.pi)

N_CORES = 8
B_TOTAL = 65536
B_CORE = B_TOTAL // N_CORES  # 8192
P = 128

DEFAULT_OPTS = dict(
    macro=4,        # groups per macro-tile
    dve_mul_cols=88,     # of each 196-col cumprod mul, cols given to DVE
    copy_act_cols=0,     # ET-copy columns per pair given to ACT (rest DVE)
    pair=2,         # groups sharing one PSUM transpose tile + one copy
    x_bufs=8, et_bufs=3, pt_bufs=3,
    gpb=(51, 13),   # groups per logits bank (51*10*4B = 2040B: fills a bank)
    bank_lag=2,     # macros between a bank's last matmul and its bias-add
    tail_act_macros=0,   # trailing macros whose ET copies ride ACT (drain)
    dma_split=1,    # X DMAs per macro
    scalar_dma=(),       # ACT-ring DMA pre-Sin would trigger a set-0 table load
    gpsimd_dma=(1, 2, 3),  # fill macros via SWDGE: parallel DMA issue lane
    head_taper=(1, 1, 1, 1, 2, 2, 2, 2),  # graded fill: DMA-paced start
    taper=(2,),                     # small macro last: fast drain
    stt_pool=False,      # ILLEGAL on hw: walrus rejects TensorScalarPtr on Pool
    dma_copy_every=0,    # every Nth pair-copy rides a DMA instead of DVE
    pool_sub=False,      # softmax subtract on Pool instead of DVE
    pool_reduce=False,   # (unsupported: Pool reduce is partition-axis only)
    mm_lag=0,       # DO NOT ENABLE: deferred-matmul emission races (wrong
                    # results / inf nondeterministically in multi-core runs)
    tail_act_pairs=1,    # last pair-copy rides the post-Sin idle ACT
    out_sync_dma=True,   # output DMAs on the (drain-idle) SP ring
    macro_batch=False,   # phase-batched tails: no sim effect, keep simple
    mm_bias=True,   # bias = 1^T @ b as the opening matmul accumulation member
    bank_chain=True,     # full per-bank softmax chains at the drain
)


def build(groups: int, opts: dict | None = None):
    o = dict(DEFAULT_OPTS)
    if opts:
        o.update(opts)
    macro = o["macro"]
    assert groups % macro == 0
    b_core = groups * P

    nc = bacc.Bacc("TRN2", target_bir_lowering=False, debug=False,
                   num_devices=N_CORES)

    xin = nc.dram_tensor("x", [b_core, 784], BF16, kind="ExternalInput").ap()
    wt_in = nc.dram_tensor("wt", [112, 70], BF16, kind="ExternalInput").ap()
    bh_in = nc.dram_tensor("bh", [P, 10], F32, kind="ExternalInput").ap()
    id_in = nc.dram_tensor("ident", [P, P], BF16, kind="ExternalInput").ap()
    one_in = nc.dram_tensor("one", [1, P], BF16, kind="ExternalInput").ap()
    bb_in = nc.dram_tensor("bb", [1, 10], BF16, kind="ExternalInput").ap()
    y = nc.dram_tensor("y", [b_core, 10], F32, kind="ExternalOutput").ap()

    with tile.TileContext(nc) as tc, ExitStack() as ctx:
        const = ctx.enter_context(tc.tile_pool(name="const", bufs=1))
        xpool = ctx.enter_context(tc.tile_pool(name="xp", bufs=o["x_bufs"]))
        etpool = ctx.enter_context(tc.tile_pool(name="et", bufs=o["et_bufs"]))
        spool = ctx.enter_context(tc.tile_pool(name="sm", bufs=1))
        pt_ps = ctx.enter_context(
            tc.tile_pool(name="pt", bufs=o["pt_bufs"], space="PSUM"))
        lg_ps = ctx.enter_context(
            tc.tile_pool(name="lg", bufs=1, space="PSUM"))

        # const loads are emitted inside emit_all after the first X tile's
        # DMA, so neither SP's FIFO nor ACT's sequencer delays the pipeline
        WT = const.tile([112, 70], BF16)
        BH = const.tile([P, 10], F32)
        ID = const.tile([P, P], BF16)
        # ones row + bias row: bias folded into the matmul accumulation
        # (out = 1^T @ b + sum_c ET_c^T @ WT_c), so no DVE bias-add pass
        ONE = const.tile([1, P], BF16)
        BB = const.tile([1, 10], BF16)

        def emit_consts():
            nc.sync.dma_start(WT[:], wt_in[:, :])
            if o.get("mm_bias"):
                nc.sync.dma_start(ONE[:], one_in[:, :])
                nc.sync.dma_start(BB[:], bb_in[:, :])
            else:
                nc.sync.dma_start(BH[:], bh_in[:, :])
            nc.sync.dma_start(ID[:], id_in[:, :])

        # macro schedule with optional tapers for short fill + drain
        macros = [macro] * (groups // macro)
        head = tuple(o.get("head_taper") or ())
        tail = tuple(o.get("taper") or ())
        while head and (sum(head) % macro or sum(head) // macro >= len(macros)):
            head = head[:-1]
        if head:
            macros = list(head) + macros[sum(head) // macro:]
        nfull = sum(1 for v in macros if v == macro)
        while tail and (sum(tail) % macro or sum(tail) // macro >= nfull):
            tail = tail[:-1]
        if tail:
            macros = macros[:len(macros) - sum(tail) // macro] + list(tail)
        mid = o.get("mid_macro", 0)
        if mid > macro:
            # coalesce runs of full macros into bigger mid-stream macros:
            # fewer Sin/mul instructions (less per-instruction overhead)
            out = []
            run = 0
            for v in macros + [None]:
                if v == macro:
                    run += macro
                    if run == mid:
                        out.append(mid)
                        run = 0
                else:
                    out.extend([macro] * (run // macro))
                    run = 0
                    if v is not None:
                        out.append(v)
            macros = out
        assert sum(macros) == groups
        starts = [sum(macros[:i]) for i in range(len(macros))]
        n_macro = len(macros)
        total_pairs = [sum(-(-mv // min(o["pair"], mv)) for mv in macros)]

        # logits stay resident in PSUM until the softmax tail; per-bank
        # softmax chains are emitted as soon as a bank's matmuls complete so
        # they interleave with later macros (the Tile schedule is static per
        # engine).  A small last bank keeps the drain chain short.
        gpb = o.get("gpb", 16)
        if isinstance(gpb, int):
            banks = []
            left = groups
            while left > 0:
                banks.append(min(gpb, left))
                left -= gpb
        else:
            banks = list(gpb)
        assert sum(banks) == groups
        bank_start = [sum(banks[:i]) for i in range(len(banks))]
        LGS = [lg_ps.tile([P, banks[i] * 10], F32, name=f"LG{i}", tag=f"LG{i}")
               for i in range(len(banks))]

        def bank_of(g):
            for i in range(len(banks)):
                if g < bank_start[i] + banks[i]:
                    return i
            raise AssertionError

        def lg_slice(g):
            i = bank_of(g)
            j = g - bank_start[i]
            return LGS[i][:, j * 10:j * 10 + 10]

        xt = {}

        # sample s of this core lives at partition s // groups, group
        # s % groups: the output rows per partition are then CONTIGUOUS in y
        # (1920B runs instead of scattered 40B runs -> ~3x faster out-DMA).
        # Input runs stay 1568B/partition, so input DMA efficiency is equal.
        xv = xin.rearrange("(p g) q -> p g q", p=P)

        def emit_dma(m):
            macro = macros[m]
            X = xpool.tile([P, macro * 784], BF16)
            # early macros listed in scalar_dma ride the ACT HWDGE ring so
            # their transfers overlap dma(0)'s on the SP ring (faster fill);
            # gpsimd_dma macros use the SWDGE path (third issue stream)
            if m in o.get("gpsimd_dma", ()):
                eng = nc.gpsimd
            elif m in o.get("scalar_dma", ()):
                eng = nc.scalar
            elif m in o.get("vector_dma", ()):
                eng = nc.vector
            else:
                eng = nc.sync
            if m == 0 and macro == 1 and o.get("head_half_dma"):
                # two half-group transfers on separate rings: halves the
                # first tile's transfer latency so Sin0a starts earlier
                g = starts[0]
                nc.sync.dma_start(X[:, 0:392], xv[:, g, 0:392])
                nc.gpsimd.dma_start(X[:, 392:784], xv[:, g, 392:784])
                xt[m] = X
                return
            ds = min(o["dma_split"], macro)
            step = macro // ds
            for k in range(ds):
                g = starts[m] + k * step
                eng.dma_start(
                    X[:, 784 * k * step:784 * (k + 1) * step].rearrange(
                        "p (s q) -> p s q", s=step),
                    xv[:, g:g + step, :])
            xt[m] = X

        def emit_front(m):
            macro = macros[m]
            X = xt[m]
            # cos x = sin(wrap(x + pi/2)); host shipped the wrapped angles in
            # plane order, so one contiguous in-place Sin covers the macro.
            if m == 0 and macro == 1 and o.get("head_split", False):
                nc.scalar.activation(X[:, 0:392], X[:, 0:392], AF.Sin)
                nc.scalar.activation(X[:, 392:784], X[:, 392:784], AF.Sin)
            else:
                nc.scalar.activation(X[:], X[:], AF.Sin)
            cpl = X[:].rearrange("p (g pl q) -> p g pl q", g=macro, pl=4,
                                 q=196)
            cd = o["dve_mul_cols"]
            if n_macro - m <= o.get("tail_dve_macros", 0):
                cd = 196   # drain region: whole muls on DVE, no pool gating
            tdc = o.get("tail_dve_cols")
            if tdc and n_macro - m <= tdc[0]:
                cd = tdc[1]  # graded drain split: shorter Pool chains at the
                # end without dumping the whole mul load on DVE's queue
            cds = cd if isinstance(cd, (list, tuple)) else (cd, cd, cd)
            for j in range(3):
                c = cds[j]
                if c > 0:
                    nc.vector.tensor_mul(cpl[:, :, j + 1, 0:c],
                                         cpl[:, :, j, 0:c],
                                         cpl[:, :, j + 1, 0:c])
                if c < 196:
                    if o.get("stt_pool"):
                        # TensorScalarPtr opcode: 0.60 gpsimd efficiency vs
                        # TensorTensor-mult's 0.42 -> 1.43x faster Pool muls
                        nc.gpsimd.scalar_tensor_tensor(
                            cpl[:, :, j + 1, c:196],
                            cpl[:, :, j, c:196],
                            1.0,
                            cpl[:, :, j + 1, c:196],
                            op0=mybir.AluOpType.mult,
                            op1=mybir.AluOpType.mult)
                    else:
                        nc.gpsimd.tensor_mul(cpl[:, :, j + 1, c:196],
                                             cpl[:, :, j, c:196],
                                             cpl[:, :, j + 1, c:196])

        pair_seq = [0]  # running pair index for tail_act_pairs
        pt_by_k0 = {}   # macro-local PT/ET handles for macro_batch phases
        et_by_k0 = {}

        def emit_tail(m):
            macro = macros[m]
            C4 = xt.pop(m)
            zc = o["copy_act_cols"]
            if n_macro - m <= o.get("tail_act_macros", 0):
                zc = 1 << 30   # whole-pair copies on ACT in the drain region
            pair0 = min(o["pair"], macro)
            batch = o.get("macro_batch", False)
            phases = ([("T",), ("C",), ("M",)] if batch
                      else [("T", "C", "M")])
            for todo in phases:
                for k0 in range(0, macro, pair0):
                    pair = min(pair0, macro - k0)
                    if "T" in todo:
                        PT = pt_ps.tile([112, pair * 7 * P], BF16, tag="PT")
                        pt_by_k0[k0] = PT
                        for kk in range(pair):
                            k = k0 + kk
                            for c in range(7):
                                nc.tensor.transpose(
                                    PT[:, P * (7 * kk + c):P * (7 * kk + c + 1)],
                                    C4[:, 784 * k + 112 * c:784 * k + 112 * (c + 1)],
                                    ID[:])
                    if "C" in todo:
                        PT = pt_by_k0[k0]
                        ET = etpool.tile([112, pair * 7 * P], BF16, tag="ET")
                        et_by_k0[k0] = ET
                        zce = min(zc, pair * 7 * P)
                        pair_seq[0] += 1
                        tap = o.get("tail_act_pairs", 0)
                        if tap and total_pairs[0] - pair_seq[0] < tap:
                            zce = pair * 7 * P  # last K pair-copies: idle ACT
                        if zce >= pair * 7 * P and pair > 1:
                            # per-group copies: the first group's matmuls start
                            # while the second group's copy still runs (drain)
                            for kk in range(pair):
                                nc.scalar.copy(
                                    ET[:, 7 * P * kk:7 * P * (kk + 1)],
                                    PT[:, 7 * P * kk:7 * P * (kk + 1)])
                        elif zce > 0:
                            nc.scalar.copy(ET[:, 0:zce], PT[:, 0:zce])
                            if zce < pair * 7 * P:
                                nc.vector.tensor_copy(ET[:, zce:], PT[:, zce:])
                        else:
                            nc.vector.tensor_copy(ET[:], PT[:])
                    if "M" in todo:
                        ET = et_by_k0[k0]
                        for kk in range(pair):
                            g = starts[m] + k0 + kk
                            if o.get("mm_bias"):
                                # bias as opening accumulation member: 1^T @ b
                                nc.tensor.matmul(lg_slice(g), ONE[:, :],
                                                 BB[:, :], start=True,
                                                 stop=False)
                            for c in range(7):
                                nc.tensor.matmul(
                                    lg_slice(g),
                                    ET[:, P * (7 * kk + c):P * (7 * kk + c + 1)],
                                    WT[:, 10 * c:10 * (c + 1)],
                                    start=(c == 0) and not o.get("mm_bias"),
                                    stop=(c == 6))

        lt = spool.tile([P, groups * 10], F32)
        ex = spool.tile([P, groups * 10], F32)
        sums = spool.tile([P, groups], F32)
        lns = spool.tile([P, groups], F32)
        outp = spool.tile([P, groups * 10], F32)
        yv = y.rearrange("(p g) t -> p g t", p=P)

        def emit_bank_add(i):
            # bias add for one logits bank (reads PSUM); deps are long done
            # by emission time, so it never stalls the DVE stream
            if o.get("mm_bias"):
                return  # bias rides the matmul accumulation instead
            ng = banks[i]
            g0 = bank_start[i]
            g1 = g0 + ng
            ltb = lt[:, g0 * 10:g1 * 10]
            nc.vector.tensor_add(
                ltb.rearrange("p (g t) -> p g t", g=ng),
                LGS[i][:].rearrange("p (g t) -> p g t", g=ng),
                BH[:].unsqueeze(1).broadcast_to([P, ng, 10]))

        def emit_bank_exp(i):
            # emitted right after the final Sin: all Exp/Ln calls share one
            # natural_log_exp table load, and ready banks' exps fill ACT's
            # idle window while the last macros' tails still run
            ng = banks[i]
            g0 = bank_start[i]
            g1 = g0 + ng
            src = LGS[i][:] if o.get("mm_bias") else lt[:, g0 * 10:g1 * 10]
            nc.scalar.activation(ex[:, g0 * 10:g1 * 10], src, AF.Exp)

        def emit_bank_tail(i):
            # reduce/ln/sub/dma for one bank
            ng = banks[i]
            g0 = bank_start[i]
            g1 = g0 + ng
            if o.get("mm_bias"):
                ltb = LGS[i][:]
            else:
                ltb = lt[:, g0 * 10:g1 * 10]
            exb = ex[:, g0 * 10:g1 * 10]
            red_eng = nc.gpsimd if o.get("pool_reduce") else nc.vector
            red_eng.reduce_sum(sums[:, g0:g1],
                               exb.rearrange("p (g t) -> p g t", g=ng),
                               axis=mybir.AxisListType.X)
            nc.scalar.activation(lns[:, g0:g1], sums[:, g0:g1], AF.Ln)
            sub_eng = nc.gpsimd if o.get("pool_sub") else nc.vector
            sub_eng.tensor_sub(
                outp[:, g0 * 10:g1 * 10].rearrange("p (g t) -> p g t", g=ng),
                ltb.rearrange("p (g t) -> p g t", g=ng),
                lns[:, g0:g1].unsqueeze(2).broadcast_to([P, ng, 10]))
            # scalar-issued HWDGE keeps output DMAs out of SP's FIFO, so a
            # dep-blocked output never stalls later input prefetches; at the
            # drain SP is idle, so out_sync_dma can route them there instead
            out_eng = nc.sync if o.get("out_sync_dma") else nc.scalar
            out_eng.dma_start(
                yv[:, g0:g1, :],
                outp[:, g0 * 10:g1 * 10].rearrange("p (g t) -> p g t", g=ng))

        def emit_all():
            # software-pipelined emission: dma(t) | front(t-1) | tail(t-2);
            # bank softmax chains are emitted `bank_lag` macros after their
            # groups' matmuls so the (in-order) engine streams never stall on
            # a not-yet-satisfied dependency.
            lag = o.get("bank_lag", 2)
            bank_ready = {}
            for m in range(n_macro):
                done = starts[m] + macros[m]
                for i in range(len(banks)):
                    if bank_start[i] + banks[i] <= done and i not in bank_ready:
                        bank_ready[i] = m
            next_bank = 0
            exps_done = 0
            for t in range(n_macro + 2 + lag):
                if t < n_macro:
                    emit_dma(t)
                if t == o.get("const_t", 0):
                    emit_consts()
                if 1 <= t <= n_macro:
                    emit_front(t - 1)
                if t == n_macro:
                    # last Sin just emitted: queue ready banks' exps now so
                    # they precede the drain-region ACT copies in the FIFO
                    while exps_done < next_bank:
                        emit_bank_exp(exps_done)
                        exps_done += 1
                if 2 <= t < n_macro + 2:
                    emit_tail(t - 2)
                while (next_bank < len(banks)
                       and t - 2 - lag >= bank_ready.get(next_bank, 1 << 30)):
                    emit_bank_add(next_bank)
                    next_bank += 1
            while next_bank < len(banks):
                emit_bank_add(next_bank)
                next_bank += 1
            if o.get("bank_chain"):
                # full per-bank chains: a later bank's exp sits after an
                # earlier bank's ln in the ACT queue, but that ln's reduce is
                # already in flight, so nothing dep-blocks the FIFO
                for i in range(len(banks)):
                    if i >= exps_done:
                        emit_bank_exp(i)
                    emit_bank_tail(i)
            else:
                while exps_done < len(banks):
                    emit_bank_exp(exps_done)
                    exps_done += 1
                for i in range(len(banks)):
                    emit_bank_tail(i)

        rep = o.get("repeat", 1)
        if rep > 1:
            with tc.For_i(0, rep, 1,
                          hint_engines=(mybir.EngineType.PE,
                                        mybir.EngineType.Activation,
                                        mybir.EngineType.DVE)):
                emit_all()
        else:
            emit_all()

    nc.compile()
    return nc


def host_x(x):
    """Plane-permute + wrap on host: a = wrap(x + pi/2) into [-pi, pi], in
    group order [pl(4), r(14), c(14)] per sample (pl = 2*jr + jc), bf16.

    cos(x) = sin(a) exactly; the device then needs a single contiguous Sin.
    """
    x = np.asarray(x, dtype=np.float32).reshape(-1, 28, 28)
    xp = x.reshape(-1, 14, 2, 14, 2).transpose(0, 2, 4, 1, 3)  # b,jr,jc,r,c
    a = np.mod(xp + (PI / 2 + PI), 2 * PI, dtype=np.float32) - PI
    return {"x": np.ascontiguousarray(a).reshape(-1, 784).astype(ml_dtypes.bfloat16)}


def host_inputs(W, b):
    """Permuted/bf16 weight chunks + broadcast bias + identity.

    Within a group, feature q' = 196*pl + (14*r + c) maps to original W
    column 4*(14*r+c) + pl.  Chunk c' = rows [112c', 112c'+112).
    """
    W = np.asarray(W, dtype=np.float32)
    b = np.asarray(b, dtype=np.float32)
    qp = np.arange(784)
    pl, p = qp // 196, qp % 196
    wperm = W[:, 4 * p + pl]                    # [10, 784] block order
    wt = np.zeros((112, 70), dtype=np.float32)
    for c in range(7):
        wt[:, 10 * c:10 * (c + 1)] = wperm[:, 112 * c:112 * (c + 1)].T
    return {
        "wt": wt.astype(ml_dtypes.bfloat16),
        "bh": np.tile(b[None, :], (P, 1)).astype(np.float32),
        "ident": np.eye(P, dtype=np.float32).astype(ml_dtypes.bfloat16),
        "one": np.ones((1, P), dtype=np.float32).astype(ml_dtypes.bfloat16),
        "bb": b[None, :].astype(ml_dtypes.bfloat16),
    }


_NC_CACHE = {}


def kernel(x, W, b):
    xs = host_x(x)["x"]
    key = B_CORE // P
    if key not in _NC_CACHE:
        _NC_CACHE[key] = build(groups=key)
    nc = _NC_CACHE[key]
    shared = host_inputs(W, b)
    in_maps = [
        {"x": xs[i * B_CORE:(i + 1) * B_CORE], **shared} for i in range(N_CORES)
    ]
    res = run_bass_kernel_spmd(nc, in_maps, list(range(N_CORES)))
    return np.concatenate([res.results[i]["y"] for i in range(N_CORES)], axis=0)


if __name__ == "__main__":
    rng = np.random.default_rng(0)
    x = rng.standard_normal((B_TOTAL, 1, 28, 28), dtype=np.float32)
    W = (rng.standard_normal((10, 784)) * 0.03).astype(np.float32)
    b = np.zeros(10, np.float32)
    out = kernel(x, W, b)
    print("out", out.shape, out.dtype)

